# revision 14
# baseline (speedup 1.0000x reference)
"""Self-contained Trainium2 kernel for nn_AMDOptimizedAttention.

Reference computes, for B=2, S=2048, H=2048, nh=16, hd=128:
    q/k/v = hs @ w{q,k,v}.T  (torch Linear convention)
    q, k  = rope(q), rope(k)
    out   = causal_softmax(q @ k.T / sqrt(hd)) @ v
    y     = out @ wo.T

Sharding (Megatron-style tensor parallel over heads + data parallel over
batch): core c handles batch c//4, heads 4*(c%4) .. 4*(c%4)+3.  Each core
computes a partial y for its batch (row-sharded wo); host sums the 4
partials per batch.

v3 layout: fp16 staging everywhere (same PE speed as bf16, 8x the
mantissa), f32 PSUM accumulation.
  - scores computed transposed [k, q]; causal mask applied as a 0/1
    fp16 multiply AFTER exp (2x DVE mode, off the PSUM critical path);
    softmax denominator via a ones-column matmul accumulated in PSUM;
    1/sum via reciprocal_approx_fast.
  - per-head software pipeline: attention for head h is emitted before
    projections for head h+1; the tile scheduler fills exp-latency
    stalls on PE with projection matmuls.  PSUM budget: 4 banks for
    projections (2 tags x 2 bufs) + 4 for attention (2 score bufs +
    1 out + 1 denom).
  - output projection keeps each ao slice stationary across 4 matmuls
    (ldweights amortized), PSUM drains via scalar-engine copies, y is
    written back in fp16 (host accumulates partials in f32).
"""

import sys

if "/opt/trn_rl_repo" not in sys.path:
    sys.path.insert(0, "/opt/trn_rl_repo")

import numpy as np

B, S, H = 2, 2048, 2048
NH, HD = 16, 128
P = 128
NCORES = 8
HPC = 4              # heads per core
DSL = HPC * HD       # 512: per-core slice of the hidden dim
KO = H // P          # 16 contraction chunks for projections
TBP = 512            # projection token-block
QB = 512             # attention query-block
NQB = S // QB        # 4
SCALE = 1.0 / np.sqrt(HD)
ROPE_BASE = 10000.0

_CACHE = {}


def _build_nc():
    import concourse.mybir as mybir
    from concourse import bacc
    from concourse.tile import TileContext

    f32 = mybir.dt.float32
    f16 = mybir.dt.float16
    Alu = mybir.AluOpType
    Act = mybir.ActivationFunctionType

    nc = bacc.Bacc("TRN2", target_bir_lowering=False)

    xT = nc.declare_dram_parameter("xT", [H, S], f16, isOutput=False)
    wqT = nc.declare_dram_parameter("wqT", [H, DSL], f16, isOutput=False)
    wkT = nc.declare_dram_parameter("wkT", [H, DSL], f16, isOutput=False)
    wvT = nc.declare_dram_parameter("wvT", [H, DSL], f16, isOutput=False)
    woT = nc.declare_dram_parameter("woT", [DSL, H], f16, isOutput=False)
    # rope tables packed [128, S]: rows 0:64 cos, rows 64:128 cos (dup);
    # csb likewise for sin
    csa = nc.declare_dram_parameter("csa", [P, S], f16, isOutput=False)
    csb = nc.declare_dram_parameter("csb", [P, S], f16, isOutput=False)
    maskp = nc.declare_dram_parameter("mask", [QB // P, P, QB], f16, isOutput=False)
    onesb = nc.declare_dram_parameter("onesb", [P, P], f16, isOutput=False)
    yout = nc.declare_dram_parameter("out", [S, H], f16, isOutput=True)

    xTr = xT.rearrange("(ko p) t -> p ko t", p=P)
    wT = {"q": wqT, "k": wkT, "v": wvT}
    wTr = {k: v.rearrange("(ko p) d -> p ko d", p=P) for k, v in wT.items()}

    def mm(ps, lhsT, rhs, start, stop):
        nc.tensor.matmul(ps, lhsT, rhs, start=start, stop=stop)

    with TileContext(nc) as tc, nc.allow_low_precision(
        reason="fp16 staging is deliberate; matmuls accumulate in f32 PSUM"
    ):
        with (
            tc.tile_pool(name="res", bufs=1) as rpool,
            tc.tile_pool(name="xres", bufs=1) as xpool,
            tc.tile_pool(name="wvpool", bufs=1) as wvpool,
            tc.tile_pool(name="wstream", bufs=2) as wpool,
            tc.tile_pool(name="ropetmp", bufs=2) as rtpool,
            tc.tile_pool(name="et", bufs=6) as epool,
            tc.tile_pool(name="nrm", bufs=3) as npool,
        ):
            # ---- residents (DMA issue order = priority order) ----
            TC = S // 4
            xs = [xpool.tile([P, KO, TC], f16, tag=f"xs{g}", name=f"xs{g}")
                  for g in range(4)]            # x.T, token-column chunks
            qT = [rpool.tile([P, S], f16, tag=f"qT{h}", name=f"qT{h}")
                  for h in range(HPC)]
            kT = [rpool.tile([P, S], f16, tag=f"kT{h}", name=f"kT{h}")
                  for h in range(HPC)]
            vs = rpool.tile([P, KO, DSL], f16, tag="vs", name="vs")
            ao = [rpool.tile([P, S], f16, tag=f"ao{h}", name=f"ao{h}")
                  for h in range(HPC)]
            wv = wvpool.tile([P, KO, DSL], f16, tag="wv", name="wv")

            def wph_load(pj, h):
                t = wpool.tile([P, KO, P], f16, tag="wph", name=f"w{pj}{h}")
                nc.sync.dma_start(t[:], wTr[pj][:, :, h * P:(h + 1) * P])
                return t

            # head-0 weights + first x chunks first: PE starts after ~2.6MB;
            # wv early so v-proj matmuls can fill later x-stream stalls
            wk0 = wph_load("k", 0)
            nc.sync.dma_start(xs[0][:], xTr[:, :, 0:TC])
            wq0 = wph_load("q", 0)
            nc.sync.dma_start(xs[1][:], xTr[:, :, TC:2 * TC])
            nc.sync.dma_start(wv[:], wTr["v"][:])
            for g in range(2, 4):
                nc.sync.dma_start(xs[g][:], xTr[:, :, g * TC:(g + 1) * TC])
            csA = rpool.tile([P, S], f16, tag="csA", name="csA")
            nc.sync.dma_start(csA[:], csa[:])
            csB = rpool.tile([P, S], f16, tag="csB", name="csB")
            nc.sync.dma_start(csB[:], csb[:])
            masks = rpool.tile([P, QB // P, QB], f16, tag="masks", name="masks")
            nc.sync.dma_start(masks[:], maskp.rearrange("j p f -> p j f"))
            oneb = rpool.tile([P, P], f16, tag="oneb", name="oneb")
            nc.sync.dma_start(oneb[:], onesb[:])
            wos = rpool.tile([P, DSL // P, H], f16, tag="wos", name="wos")
            nc.sync.dma_start(wos[:], woT.rearrange("(ko p) e -> p ko e", p=P))

            def xsl(ko, t0, t1):
                g = t0 // TC
                assert t1 <= (g + 1) * TC
                return xs[g][:, ko, t0 - g * TC:t1 - g * TC]

            # ---- k/q projection, one token-block at a time ----
            NTB = S // TBP

            def proj_head(pj, h, wph, dst, tbs=None):
                for tb in (range(NTB) if tbs is None else tbs):
                    ps = psqk.tile([P, TBP], f32, tag="psqk",
                                   name="psqk", bufs=2)
                    for ko in range(KO):
                        mm(ps, wph[:, ko, :],
                           xsl(ko, tb * TBP, (tb + 1) * TBP),
                           ko == 0, ko == KO - 1)
                    tslc = slice(tb * TBP, (tb + 1) * TBP)
                    # m1 = [x1*cos; x2*cos], m2 = [x2*sin; x1*sin] (halves
                    # swapped at creation: PSUM source is exempt from the
                    # same-start-partition rule, SBUF operands stay aligned;
                    # the final sub/add are SBUF-only fp16 and run on GpSimd)
                    m1 = rtpool.tile([P, TBP], f16, tag="m1", name="m1")
                    m2 = rtpool.tile([P, TBP], f16, tag="m2", name="m2")
                    nc.vector.tensor_tensor(m1[:], ps[:], csA[:, tslc], Alu.mult)
                    nc.vector.tensor_tensor(
                        m2[0:64, :], ps[64:128, :], csB[0:64, tslc], Alu.mult)
                    nc.vector.tensor_tensor(
                        m2[64:128, :], ps[0:64, :], csB[64:128, tslc], Alu.mult)
                    nc.gpsimd.tensor_tensor(
                        dst[0:64, tslc], m1[0:64, :], m2[0:64, :], Alu.subtract)
                    nc.gpsimd.tensor_tensor(
                        dst[64:128, tslc], m1[64:128, :], m2[64:128, :], Alu.add)

            # ---- attention for one head-qb (scores transposed [k, q]) ----
            # softmax denominator: et tiles are accumulated elementwise on
            # DVE (fp16 2x mode) into esum; ONE ones-matmul per (h, qb)
            # partition-reduces esum into pd.
            def attn_head_qb(h, qb):
                qsl = slice(qb * QB, (qb + 1) * QB)
                nkt = (qb + 1) * (QB // P)
                po = pso.tile([P, QB], f32, tag="po", name="po")
                esum = npool.tile([P, QB], f16, tag="esum", name="esum", bufs=2)
                for kt in range(nkt):
                    pscr = pss.tile([P, QB], f32, tag="pscr", name="pscr")
                    mm(pscr, kT[h][:, kt * P:(kt + 1) * P],
                       qT[h][:, qsl], True, True)
                    et = epool.tile([P, QB], f16, tag="et", name="et")
                    nc.scalar.activation(et[:], pscr[:], Act.Exp,
                                         scale=float(SCALE))
                    j = kt - qb * (QB // P)
                    if j >= 0:
                        nc.gpsimd.tensor_tensor(
                            et[:], et[:], masks[:, j, :], Alu.mult
                        )
                    if kt == 0:
                        nc.vector.tensor_copy(esum[:], et[:])
                    else:
                        nc.vector.tensor_tensor(esum[:], esum[:], et[:], Alu.add)
                    mm(po, vs[:, kt, h * P:(h + 1) * P], et[:],
                       kt == 0, kt == nkt - 1)
                pd = psd.tile([P, QB], f32, tag="pd", name="pd")
                mm(pd, oneb[:], esum[:], True, True)
                rec = npool.tile([P, QB], f32, tag="rec", name="rec")
                nc.vector.reciprocal_approx_fast(rec[:], pd[:])
                nc.vector.tensor_tensor(
                    ao[h][:, qsl], po[:], rec[:], Alu.mult
                )

            def attn_head(h):
                for qb in range(NQB):
                    attn_head_qb(h, qb)

            # ---- proj + attention share one 8-bank PSUM budget ----
            NEC = H // QB
            with (
                tc.tile_pool(name="pss", bufs=3, space="PSUM") as pss_,
                tc.tile_pool(name="pso", bufs=2, space="PSUM") as pso_,
                tc.tile_pool(name="psd", bufs=1, space="PSUM") as psd_,
                tc.tile_pool(name="ystage", bufs=2) as ypool,
            ):
                pss, pso, psd = pss_, pso_, psd_

                with tc.tile_pool(name="psqk", bufs=1, space="PSUM") as psqk_:
                    psqk = psqk_

                    # k0/q0/v interleaved per x-chunk: each 2.1MB chunk
                    # unlocks ~20us of PE work, so the x stream never
                    # starves the startup (pv shares the score-bank ring)
                    for g in range(4):
                        proj_head("k", 0, wk0, kT[0], tbs=[g])
                        proj_head("q", 0, wq0, qT[0], tbs=[g])
                        for tt in range(4 * g, 4 * g + 4):
                            pv = pss.tile([P, DSL], f32, tag="pscr", name="pv")
                            for ko in range(KO):
                                mm(pv, xsl(ko, tt * P, (tt + 1) * P),
                                   wv[:, ko, :], ko == 0, ko == KO - 1)
                            nc.scalar.copy(vs[:, tt, :], pv[:])

                    # per-head pipeline: attention(h) fills with proj(h+1)
                    for h in range(HPC - 1):
                        attn_head(h)
                        wk = wph_load("k", h + 1)
                        proj_head("k", h + 1, wk, kT[h + 1])
                        wq = wph_load("q", h + 1)
                        proj_head("q", h + 1, wq, qT[h + 1])

                # ---- attn(3) interleaved with output projection ----
                # psqk's 4 banks recycle into psy; oproj tiles for token
                # block qb unlock as soon as attn(3, qb) normalizes.
                with tc.tile_pool(name="psy", bufs=1, space="PSUM") as psy:
                    def oproj_tile(tt):
                        tsl = slice(tt * P, (tt + 1) * P)
                        yst = ypool.tile([P, H], f16, tag="yst", name="yst")
                        for half in range(NEC // 2):
                            pys = [psy.tile([P, QB], f32, tag=f"py{i}",
                                            name=f"py{i}") for i in range(2)]
                            for dc in range(DSL // P):
                                for i in range(2):
                                    mm(pys[i], ao[dc][:, tsl],
                                       wos[:, dc, (2 * half + i) * QB:
                                           (2 * half + i + 1) * QB],
                                       dc == 0, dc == DSL // P - 1)
                            for i in range(2):
                                ec = 2 * half + i
                                eng_copy = (nc.scalar.copy if i == 0
                                            else nc.vector.tensor_copy)
                                eng_copy(yst[:, ec * QB:(ec + 1) * QB],
                                         pys[i][:])
                        nc.sync.dma_start(yout[tsl, :], yst[:])

                    for qb in range(NQB):
                        attn_head_qb(HPC - 1, qb)
                        for tt in range(qb * NQB, (qb + 1) * NQB):
                            oproj_tile(tt)

    nc.finalize()
    return nc


def _host_inputs(hidden_states, wq, wk, wv, wo):
    f32 = np.float32
    f16 = np.float16
    ca = np.ascontiguousarray

    inv = 1.0 / (ROPE_BASE ** (np.arange(0, HD, 2, dtype=f32) / HD))
    t = np.arange(S, dtype=f32)
    fr = np.outer(t, inv)                      # [S, 64]
    cosT = np.cos(fr).T.astype(f32)            # [64, S]
    sinT = np.sin(fr).T.astype(f32)
    csa = ca(np.concatenate([cosT, cosT], axis=0)).astype(f16)  # [128, S]
    csb = ca(np.concatenate([sinT, sinT], axis=0)).astype(f16)

    jj, pp, ff = np.meshgrid(
        np.arange(QB // P), np.arange(P), np.arange(QB), indexing="ij"
    )
    mask = np.where(jj * P + pp > ff, f16(0.0), f16(1.0)).astype(f16)
    onesb = np.ones((P, P), f16)

    xTb = [ca(hidden_states[b].T.astype(f16)) for b in range(B)]

    in_maps = []
    for c in range(NCORES):
        b, hg = divmod(c, NCORES // B)
        dsl = slice(hg * DSL, (hg + 1) * DSL)
        in_maps.append({
            "xT": xTb[b],
            "wqT": ca(wq[dsl, :].T.astype(f16)),
            "wkT": ca(wk[dsl, :].T.astype(f16)),
            "wvT": ca(wv[dsl, :].T.astype(f16)),
            "woT": ca(wo[:, dsl].T.astype(f16)),
            "csa": csa, "csb": csb,
            "mask": mask, "onesb": onesb,
        })
    return in_maps


def kernel(hidden_states, wq, wk, wv, wo, trace=False):
    from concourse.bass_utils import run_bass_kernel_spmd

    if "nc" not in _CACHE:
        _CACHE["nc"] = _build_nc()
    nc = _CACHE["nc"]

    in_maps = _host_inputs(
        np.asarray(hidden_states), np.asarray(wq), np.asarray(wk),
        np.asarray(wv), np.asarray(wo),
    )
    res = run_bass_kernel_spmd(nc, in_maps, core_ids=list(range(NCORES)),
                               trace=trace)
    y = np.zeros((B, S, H), np.float32)
    for c in range(NCORES):
        y[c // (NCORES // B)] += res.results[c]["out"].astype(np.float32)
    if trace:
        return y, res
    return y


# revision 15
# speedup vs baseline: 1.3337x; 1.3337x over previous
"""Self-contained Trainium2 kernel for nn_AMDOptimizedAttention.

Reference computes, for B=2, S=2048, H=2048, nh=16, hd=128:
    q/k/v = hs @ w{q,k,v}.T  (torch Linear convention)
    q, k  = rope(q), rope(k)
    out   = causal_softmax(q @ k.T / sqrt(hd)) @ v
    y     = out @ wo.T

Sharding (Megatron-style tensor parallel over heads + data parallel over
batch): core c handles batch c//4, heads 4*(c%4) .. 4*(c%4)+3.  Each core
computes a partial y for its batch (row-sharded wo); host sums the 4
partials per batch.

v3 layout: fp16 staging everywhere (same PE speed as bf16, 8x the
mantissa), f32 PSUM accumulation.
  - scores computed transposed [k, q]; causal mask applied as a 0/1
    fp16 multiply AFTER exp (2x DVE mode, off the PSUM critical path);
    softmax denominator via a ones-column matmul accumulated in PSUM;
    1/sum via reciprocal_approx_fast.
  - per-head software pipeline: attention for head h is emitted before
    projections for head h+1; the tile scheduler fills exp-latency
    stalls on PE with projection matmuls.  PSUM budget: 4 banks for
    projections (2 tags x 2 bufs) + 4 for attention (2 score bufs +
    1 out + 1 denom).
  - output projection keeps each ao slice stationary across 4 matmuls
    (ldweights amortized), PSUM drains via scalar-engine copies, y is
    written back in fp16 (host accumulates partials in f32).
"""

import sys

if "/opt/trn_rl_repo" not in sys.path:
    sys.path.insert(0, "/opt/trn_rl_repo")

import numpy as np

B, S, H = 2, 2048, 2048
NH, HD = 16, 128
P = 128
NCORES = 8
HPC = 4              # heads per core
DSL = HPC * HD       # 512: per-core slice of the hidden dim
KO = H // P          # 16 contraction chunks for projections
TBP = 512            # projection token-block
QB = 512             # attention query-block
NQB = S // QB        # 4
SCALE = 1.0 / np.sqrt(HD)
ROPE_BASE = 10000.0

_CACHE = {}


def _build_nc():
    import concourse.mybir as mybir
    from concourse import bacc
    from concourse.tile import TileContext

    f32 = mybir.dt.float32
    f16 = mybir.dt.float16
    Alu = mybir.AluOpType
    Act = mybir.ActivationFunctionType

    nc = bacc.Bacc("TRN2", target_bir_lowering=False)

    xT = nc.declare_dram_parameter("xT", [H, S], f16, isOutput=False)
    wqT = nc.declare_dram_parameter("wqT", [H, DSL], f16, isOutput=False)
    wkT = nc.declare_dram_parameter("wkT", [H, DSL], f16, isOutput=False)
    wvT = nc.declare_dram_parameter("wvT", [H, DSL], f16, isOutput=False)
    woT = nc.declare_dram_parameter("woT", [DSL, H], f16, isOutput=False)
    # rope tables packed [128, S]: rows 0:64 cos, rows 64:128 cos (dup);
    # csb likewise for sin
    csa = nc.declare_dram_parameter("csa", [P, S], f16, isOutput=False)
    csb = nc.declare_dram_parameter("csb", [P, S], f16, isOutput=False)
    maskp = nc.declare_dram_parameter("mask", [QB // P, P, QB], f16, isOutput=False)
    onesb = nc.declare_dram_parameter("onesb", [P, P], f16, isOutput=False)
    yout = nc.declare_dram_parameter("out", [S, H], f16, isOutput=True)

    xTr = xT.rearrange("(ko p) t -> p ko t", p=P)
    wT = {"q": wqT, "k": wkT, "v": wvT}
    wTr = {k: v.rearrange("(ko p) d -> p ko d", p=P) for k, v in wT.items()}

    def mm(ps, lhsT, rhs, start, stop):
        nc.tensor.matmul(ps, lhsT, rhs, start=start, stop=stop)

    with TileContext(nc) as tc, nc.allow_low_precision(
        reason="fp16 staging is deliberate; matmuls accumulate in f32 PSUM"
    ):
        with (
            tc.tile_pool(name="res", bufs=1) as rpool,
            tc.tile_pool(name="xres", bufs=1) as xpool,
            tc.tile_pool(name="wvpool", bufs=1) as wvpool,
            tc.tile_pool(name="wstream", bufs=2) as wpool,
            tc.tile_pool(name="ropetmp", bufs=2) as rtpool,
            tc.tile_pool(name="et", bufs=6) as epool,
            tc.tile_pool(name="nrm", bufs=3) as npool,
        ):
            # ---- residents (DMA issue order = priority order) ----
            TC = S // 4
            xs = [xpool.tile([P, KO, TC], f16, tag=f"xs{g}", name=f"xs{g}")
                  for g in range(4)]            # x.T, token-column chunks
            qT = [rpool.tile([P, S], f16, tag=f"qT{h}", name=f"qT{h}")
                  for h in range(HPC)]
            kT = [rpool.tile([P, S], f16, tag=f"kT{h}", name=f"kT{h}")
                  for h in range(HPC)]
            vs = rpool.tile([P, KO, DSL], f16, tag="vs", name="vs")
            ao = [rpool.tile([P, S], f16, tag=f"ao{h}", name=f"ao{h}")
                  for h in range(HPC)]
            wv = wvpool.tile([P, KO, DSL], f16, tag="wv", name="wv")

            def wph_load(pj, h):
                t = wpool.tile([P, KO, P], f16, tag="wph", name=f"w{pj}{h}")
                nc.sync.dma_start(t[:], wTr[pj][:, :, h * P:(h + 1) * P])
                return t

            # head-0 weights + first x chunks first: PE starts after ~2.6MB;
            # wv early so v-proj matmuls can fill later x-stream stalls
            wk0 = wph_load("k", 0)
            nc.sync.dma_start(xs[0][:], xTr[:, :, 0:TC])
            wq0 = wph_load("q", 0)
            nc.sync.dma_start(xs[1][:], xTr[:, :, TC:2 * TC])
            nc.sync.dma_start(wv[:], wTr["v"][:])
            for g in range(2, 4):
                nc.sync.dma_start(xs[g][:], xTr[:, :, g * TC:(g + 1) * TC])
            csA = rpool.tile([P, S], f16, tag="csA", name="csA")
            nc.sync.dma_start(csA[:], csa[:])
            csB = rpool.tile([P, S], f16, tag="csB", name="csB")
            nc.sync.dma_start(csB[:], csb[:])
            masks = rpool.tile([P, QB // P, QB], f16, tag="masks", name="masks")
            nc.sync.dma_start(masks[:], maskp.rearrange("j p f -> p j f"))
            oneb = rpool.tile([P, P], f16, tag="oneb", name="oneb")
            nc.sync.dma_start(oneb[:], onesb[:])
            wos = rpool.tile([P, DSL // P, H], f16, tag="wos", name="wos")
            nc.sync.dma_start(wos[:], woT.rearrange("(ko p) e -> p ko e", p=P))

            def xsl(ko, t0, t1):
                g = t0 // TC
                assert t1 <= (g + 1) * TC
                return xs[g][:, ko, t0 - g * TC:t1 - g * TC]

            # ---- k/q projection, one token-block at a time ----
            NTB = S // TBP

            def proj_head(pj, h, wph, dst, tbs=None):
                for tb in (range(NTB) if tbs is None else tbs):
                    ps = psqk.tile([P, TBP], f32, tag="psqk",
                                   name="psqk", bufs=2)
                    for ko in range(KO):
                        mm(ps, wph[:, ko, :],
                           xsl(ko, tb * TBP, (tb + 1) * TBP),
                           ko == 0, ko == KO - 1)
                    tslc = slice(tb * TBP, (tb + 1) * TBP)
                    # m1 = [x1*cos; x2*cos], m2 = [x2*sin; x1*sin] (halves
                    # swapped at creation: PSUM source is exempt from the
                    # same-start-partition rule, SBUF operands stay aligned;
                    # the final sub/add are SBUF-only fp16 and run on GpSimd)
                    m1 = rtpool.tile([P, TBP], f16, tag="m1", name="m1")
                    m2 = rtpool.tile([P, TBP], f16, tag="m2", name="m2")
                    nc.vector.tensor_tensor(m1[:], ps[:], csA[:, tslc], Alu.mult)
                    nc.vector.tensor_tensor(
                        m2[0:64, :], ps[64:128, :], csB[0:64, tslc], Alu.mult)
                    nc.vector.tensor_tensor(
                        m2[64:128, :], ps[0:64, :], csB[64:128, tslc], Alu.mult)
                    nc.vector.tensor_tensor(
                        dst[0:64, tslc], m1[0:64, :], m2[0:64, :], Alu.subtract)
                    nc.vector.tensor_tensor(
                        dst[64:128, tslc], m1[64:128, :], m2[64:128, :], Alu.add)

            # ---- attention for one head-qb (scores transposed [k, q]) ----
            # softmax denominator: et tiles are accumulated elementwise on
            # DVE (fp16 2x mode) into esum; ONE ones-matmul per (h, qb)
            # partition-reduces esum into pd.
            def attn_head_qb(h, qb):
                qsl = slice(qb * QB, (qb + 1) * QB)
                nkt = (qb + 1) * (QB // P)
                po = pso.tile([P, QB], f32, tag="po", name="po")
                esum = npool.tile([P, QB], f16, tag="esum", name="esum", bufs=2)
                for kt in range(nkt):
                    pscr = pss.tile([P, QB], f32, tag="pscr", name="pscr")
                    mm(pscr, kT[h][:, kt * P:(kt + 1) * P],
                       qT[h][:, qsl], True, True)
                    et = epool.tile([P, QB], f16, tag="et", name="et")
                    nc.scalar.activation(et[:], pscr[:], Act.Exp,
                                         scale=float(SCALE))
                    j = kt - qb * (QB // P)
                    if j >= 0:
                        nc.vector.tensor_tensor(
                            et[:], et[:], masks[:, j, :], Alu.mult
                        )
                    if kt == 0:
                        nc.vector.tensor_copy(esum[:], et[:])
                    else:
                        nc.vector.tensor_tensor(esum[:], esum[:], et[:], Alu.add)
                    mm(po, vs[:, kt, h * P:(h + 1) * P], et[:],
                       kt == 0, kt == nkt - 1)
                pd = psd.tile([P, QB], f32, tag="pd", name="pd")
                mm(pd, oneb[:], esum[:], True, True)
                rec = npool.tile([P, QB], f32, tag="rec", name="rec")
                nc.vector.reciprocal_approx_fast(rec[:], pd[:])
                nc.vector.tensor_tensor(
                    ao[h][:, qsl], po[:], rec[:], Alu.mult
                )

            def attn_head(h):
                for qb in range(NQB):
                    attn_head_qb(h, qb)

            # ---- proj + attention share one 8-bank PSUM budget ----
            NEC = H // QB
            with (
                tc.tile_pool(name="pss", bufs=3, space="PSUM") as pss_,
                tc.tile_pool(name="pso", bufs=2, space="PSUM") as pso_,
                tc.tile_pool(name="psd", bufs=1, space="PSUM") as psd_,
                tc.tile_pool(name="ystage", bufs=2) as ypool,
            ):
                pss, pso, psd = pss_, pso_, psd_

                with tc.tile_pool(name="psqk", bufs=1, space="PSUM") as psqk_:
                    psqk = psqk_

                    # k0/q0/v interleaved per x-chunk: each 2.1MB chunk
                    # unlocks ~20us of PE work, so the x stream never
                    # starves the startup (pv shares the score-bank ring)
                    for g in range(4):
                        proj_head("k", 0, wk0, kT[0], tbs=[g])
                        proj_head("q", 0, wq0, qT[0], tbs=[g])
                        for tt in range(4 * g, 4 * g + 4):
                            pv = pss.tile([P, DSL], f32, tag="pscr", name="pv")
                            for ko in range(KO):
                                mm(pv, xsl(ko, tt * P, (tt + 1) * P),
                                   wv[:, ko, :], ko == 0, ko == KO - 1)
                            nc.scalar.copy(vs[:, tt, :], pv[:])

                    # per-head pipeline: attention(h) fills with proj(h+1)
                    for h in range(HPC - 1):
                        attn_head(h)
                        wk = wph_load("k", h + 1)
                        proj_head("k", h + 1, wk, kT[h + 1])
                        wq = wph_load("q", h + 1)
                        proj_head("q", h + 1, wq, qT[h + 1])

                # ---- attn(3) interleaved with output projection ----
                # psqk's 4 banks recycle into psy; oproj tiles for token
                # block qb unlock as soon as attn(3, qb) normalizes.
                with tc.tile_pool(name="psy", bufs=1, space="PSUM") as psy:
                    def oproj_tile(tt):
                        tsl = slice(tt * P, (tt + 1) * P)
                        yst = ypool.tile([P, H], f16, tag="yst", name="yst")
                        for half in range(NEC // 2):
                            pys = [psy.tile([P, QB], f32, tag=f"py{i}",
                                            name=f"py{i}") for i in range(2)]
                            for dc in range(DSL // P):
                                for i in range(2):
                                    mm(pys[i], ao[dc][:, tsl],
                                       wos[:, dc, (2 * half + i) * QB:
                                           (2 * half + i + 1) * QB],
                                       dc == 0, dc == DSL // P - 1)
                            for i in range(2):
                                ec = 2 * half + i
                                eng_copy = (nc.scalar.copy if i == 0
                                            else nc.vector.tensor_copy)
                                eng_copy(yst[:, ec * QB:(ec + 1) * QB],
                                         pys[i][:])
                        nc.sync.dma_start(yout[tsl, :], yst[:])

                    for qb in range(NQB):
                        attn_head_qb(HPC - 1, qb)
                        for tt in range(qb * NQB, (qb + 1) * NQB):
                            oproj_tile(tt)

    nc.finalize()
    return nc


def _host_inputs(hidden_states, wq, wk, wv, wo):
    f32 = np.float32
    f16 = np.float16
    ca = np.ascontiguousarray

    inv = 1.0 / (ROPE_BASE ** (np.arange(0, HD, 2, dtype=f32) / HD))
    t = np.arange(S, dtype=f32)
    fr = np.outer(t, inv)                      # [S, 64]
    cosT = np.cos(fr).T.astype(f32)            # [64, S]
    sinT = np.sin(fr).T.astype(f32)
    csa = ca(np.concatenate([cosT, cosT], axis=0)).astype(f16)  # [128, S]
    csb = ca(np.concatenate([sinT, sinT], axis=0)).astype(f16)

    jj, pp, ff = np.meshgrid(
        np.arange(QB // P), np.arange(P), np.arange(QB), indexing="ij"
    )
    mask = np.where(jj * P + pp > ff, f16(0.0), f16(1.0)).astype(f16)
    onesb = np.ones((P, P), f16)

    xTb = [ca(hidden_states[b].T.astype(f16)) for b in range(B)]

    in_maps = []
    for c in range(NCORES):
        b, hg = divmod(c, NCORES // B)
        dsl = slice(hg * DSL, (hg + 1) * DSL)
        in_maps.append({
            "xT": xTb[b],
            "wqT": ca(wq[dsl, :].T.astype(f16)),
            "wkT": ca(wk[dsl, :].T.astype(f16)),
            "wvT": ca(wv[dsl, :].T.astype(f16)),
            "woT": ca(wo[:, dsl].T.astype(f16)),
            "csa": csa, "csb": csb,
            "mask": mask, "onesb": onesb,
        })
    return in_maps


def kernel(hidden_states, wq, wk, wv, wo, trace=False):
    from concourse.bass_utils import run_bass_kernel_spmd

    if "nc" not in _CACHE:
        _CACHE["nc"] = _build_nc()
    nc = _CACHE["nc"]

    in_maps = _host_inputs(
        np.asarray(hidden_states), np.asarray(wq), np.asarray(wk),
        np.asarray(wv), np.asarray(wo),
    )
    res = run_bass_kernel_spmd(nc, in_maps, core_ids=list(range(NCORES)),
                               trace=trace)
    y = np.zeros((B, S, H), np.float32)
    for c in range(NCORES):
        y[c // (NCORES // B)] += res.results[c]["out"].astype(np.float32)
    if trace:
        return y, res
    return y


# revision 17
# speedup vs baseline: 1.3640x; 1.0227x over previous
"""Self-contained Trainium2 kernel for nn_AMDOptimizedAttention.

Reference computes, for B=2, S=2048, H=2048, nh=16, hd=128:
    q/k/v = hs @ w{q,k,v}.T  (torch Linear convention)
    q, k  = rope(q), rope(k)
    out   = causal_softmax(q @ k.T / sqrt(hd)) @ v
    y     = out @ wo.T

Sharding (Megatron-style tensor parallel over heads + data parallel over
batch): core c handles batch c//4, heads 4*(c%4) .. 4*(c%4)+3.  Each core
computes a partial y for its batch (row-sharded wo); host sums the 4
partials per batch.

v6 layout: fp16 staging everywhere (same PE speed as bf16, 8x the
mantissa), f32 PSUM accumulation.
  - scores computed transposed [k, q]; causal mask applied as a 0/1
    fp16 multiply AFTER exp (2x DVE mode, off the PSUM critical path);
    softmax denominator: et tiles accumulated elementwise on DVE (fp16
    2x) into esum, ONE ones-matmul per (h, qb) partition-reduces it;
    1/sum via reciprocal_approx_fast.
  - per-head software pipeline: attention(h) emitted before proj(h+1);
    the tile scheduler fills exp-latency stalls on PE with projection
    matmuls.  PSUM: proj ring 3 + score/denom ring 3 + attn-out 2 = 8.
  - attention(3) runs interleaved with the output projection under a
    second-generation PSUM split (scores 2 + out 2 + y 4), staggered so
    exp stays one query-block ahead of the oproj matmuls.
  - k0/q0/v startup is interleaved per x-chunk (first chunk split in
    half) so the x DMA stream never starves PE.
"""

import sys

if "/opt/trn_rl_repo" not in sys.path:
    sys.path.insert(0, "/opt/trn_rl_repo")

import numpy as np

B, S, H = 2, 2048, 2048
NH, HD = 16, 128
P = 128
NCORES = 8
HPC = 4              # heads per core
DSL = HPC * HD       # 512: per-core slice of the hidden dim
KO = H // P          # 16 contraction chunks for projections
TBP = 512            # projection token-block
QB = 512             # attention query-block
NQB = S // QB        # 4
SCALE = 1.0 / np.sqrt(HD)
ROPE_BASE = 10000.0

_CACHE = {}


def _build_nc():
    import concourse.mybir as mybir
    from concourse import bacc
    from concourse.tile import TileContext

    f32 = mybir.dt.float32
    f16 = mybir.dt.float16
    Alu = mybir.AluOpType
    Act = mybir.ActivationFunctionType

    nc = bacc.Bacc("TRN2", target_bir_lowering=False)

    xT = nc.declare_dram_parameter("xT", [H, S], f16, isOutput=False)
    wqT = nc.declare_dram_parameter("wqT", [H, DSL], f16, isOutput=False)
    wkT = nc.declare_dram_parameter("wkT", [H, DSL], f16, isOutput=False)
    wvT = nc.declare_dram_parameter("wvT", [H, DSL], f16, isOutput=False)
    woT = nc.declare_dram_parameter("woT", [DSL, H], f16, isOutput=False)
    # rope tables packed [128, S]: rows 0:64 cos, rows 64:128 cos (dup);
    # csb likewise for sin
    csa = nc.declare_dram_parameter("csa", [P, S], f16, isOutput=False)
    csb = nc.declare_dram_parameter("csb", [P, S], f16, isOutput=False)
    maskp = nc.declare_dram_parameter("mask", [QB // P, P, QB], f16, isOutput=False)
    onesb = nc.declare_dram_parameter("onesb", [P, P], f16, isOutput=False)
    yout = nc.declare_dram_parameter("out", [S, H], f16, isOutput=True)

    xTr = xT.rearrange("(ko p) t -> p ko t", p=P)
    wT = {"q": wqT, "k": wkT, "v": wvT}
    wTr = {k: v.rearrange("(ko p) d -> p ko d", p=P) for k, v in wT.items()}

    def mm(ps, lhsT, rhs, start, stop):
        nc.tensor.matmul(ps, lhsT, rhs, start=start, stop=stop)

    with TileContext(nc) as tc, nc.allow_low_precision(
        reason="fp16 staging is deliberate; matmuls accumulate in f32 PSUM"
    ):
        with (
            tc.tile_pool(name="res", bufs=1) as rpool,
            tc.tile_pool(name="xres", bufs=1) as xpool,
            tc.tile_pool(name="wvpool", bufs=1) as wvpool,
            tc.tile_pool(name="wstream", bufs=2) as wpool,
            tc.tile_pool(name="ropetmp", bufs=2) as rtpool,
            tc.tile_pool(name="et", bufs=6) as epool,
            tc.tile_pool(name="nrm", bufs=3) as npool,
            tc.tile_pool(name="ystage", bufs=2) as ypool,
        ):
            # ---- residents (DMA issue order = priority order) ----
            TC = S // 4
            HC = TC // 2
            # first x chunk split in half for a faster PE start
            xs0 = [xpool.tile([P, KO, HC], f16, tag=f"xs0{i}", name=f"xs0{i}")
                   for i in range(2)]
            xs = [None] + [xpool.tile([P, KO, TC], f16, tag=f"xs{g}",
                                      name=f"xs{g}") for g in range(1, 4)]
            qT = [rpool.tile([P, S], f16, tag=f"qT{h}", name=f"qT{h}")
                  for h in range(HPC)]
            kT = [rpool.tile([P, S], f16, tag=f"kT{h}", name=f"kT{h}")
                  for h in range(HPC)]
            vs = rpool.tile([P, KO, DSL], f16, tag="vs", name="vs")
            ao = [rpool.tile([P, S], f16, tag=f"ao{h}", name=f"ao{h}")
                  for h in range(HPC)]
            wv = wvpool.tile([P, KO, DSL], f16, tag="wv", name="wv")

            def wph_load(pj, h):
                t = wpool.tile([P, KO, P], f16, tag="wph", name=f"w{pj}{h}")
                nc.sync.dma_start(t[:], wTr[pj][:, :, h * P:(h + 1) * P])
                return t

            # head-0 weights + x0 halves first, wv before the x tail so
            # v-proj matmuls can fill x-stream stalls
            wk0 = wph_load("k", 0)
            nc.sync.dma_start(xs0[0][:], xTr[:, :, 0:HC])
            wq0 = wph_load("q", 0)
            nc.sync.dma_start(xs0[1][:], xTr[:, :, HC:TC])
            nc.sync.dma_start(wv[:], wTr["v"][:])
            for g in range(1, 4):
                nc.sync.dma_start(xs[g][:], xTr[:, :, g * TC:(g + 1) * TC])
            csA = rpool.tile([P, S], f16, tag="csA", name="csA")
            nc.sync.dma_start(csA[:], csa[:])
            csB = rpool.tile([P, S], f16, tag="csB", name="csB")
            nc.sync.dma_start(csB[:], csb[:])
            masks = rpool.tile([P, QB // P, QB], f16, tag="masks", name="masks")
            nc.sync.dma_start(masks[:], maskp.rearrange("j p f -> p j f"))
            oneb = rpool.tile([P, P], f16, tag="oneb", name="oneb")
            nc.sync.dma_start(oneb[:], onesb[:])
            wos = rpool.tile([P, DSL // P, H], f16, tag="wos", name="wos")
            nc.sync.dma_start(wos[:], woT.rearrange("(ko p) e -> p ko e", p=P))

            def xsl(ko, t0, t1):
                # contiguous [t0, t1) slice of x.T for contraction chunk ko;
                # returns a list of APs (the first 512-token chunk is split)
                out = []
                while t0 < t1:
                    if t0 < TC:
                        i = t0 // HC
                        e = min(t1, (i + 1) * HC)
                        out.append(xs0[i][:, ko, t0 - i * HC:e - i * HC])
                    else:
                        g = t0 // TC
                        e = min(t1, (g + 1) * TC)
                        out.append(xs[g][:, ko, t0 - g * TC:e - g * TC])
                    t0 = e
                return out

            # ---- k/q projection, one token-block at a time ----
            NTB = S // TBP

            def proj_head(pj, h, wph, dst, psqk, tbs=None):
                for tb in (range(NTB) if tbs is None else tbs):
                    # token sub-blocks aligned to x piece boundaries: tb 0
                    # is processed as two 256-token halves (its x chunk is
                    # split), later tbs as one 512-token block, so every
                    # PSUM accumulation group is a single matmul target
                    if tb == 0:
                        blocks = [(0, HC), (HC, TC)]
                    else:
                        blocks = [(tb * TBP, (tb + 1) * TBP)]
                    for t0, t1 in blocks:
                        w = t1 - t0
                        ps = psqk.tile([P, TBP], f32, tag="psqk",
                                       name="psqk", bufs=3)
                        for ko in range(KO):
                            piece = xsl(ko, t0, t1)
                            assert len(piece) == 1
                            mm(ps[:, 0:w], wph[:, ko, :], piece[0],
                               ko == 0, ko == KO - 1)
                        tslc = slice(t0, t1)
                        psw = ps[:, 0:w]
                        # m1 = [x1*cos; x2*cos], m2 = [x2*sin; x1*sin]
                        # (halves swapped at creation: PSUM source is exempt
                        # from the same-start-partition rule, SBUF operands
                        # stay aligned)
                        m1 = rtpool.tile([P, TBP], f16, tag="m1", name="m1")
                        m2 = rtpool.tile([P, TBP], f16, tag="m2", name="m2")
                        nc.vector.tensor_tensor(
                            m1[:, 0:w], psw, csA[:, tslc], Alu.mult)
                        nc.vector.tensor_tensor(
                            m2[0:64, 0:w], ps[64:128, 0:w], csB[0:64, tslc],
                            Alu.mult)
                        nc.vector.tensor_tensor(
                            m2[64:128, 0:w], ps[0:64, 0:w], csB[64:128, tslc],
                            Alu.mult)
                        nc.vector.tensor_tensor(
                            dst[0:64, tslc], m1[0:64, 0:w], m2[0:64, 0:w],
                            Alu.subtract)
                        nc.vector.tensor_tensor(
                            dst[64:128, tslc], m1[64:128, 0:w],
                            m2[64:128, 0:w], Alu.add)

            # ---- attention for one (head, query-block) ----
            def attn_head_qb(h, qb, pss, pso):
                qsl = slice(qb * QB, (qb + 1) * QB)
                nkt = (qb + 1) * (QB // P)
                po = pso.tile([P, QB], f32, tag="po", name="po")
                esum = npool.tile([P, QB], f16, tag="esum", name="esum", bufs=2)
                for kt in range(nkt):
                    pscr = pss.tile([P, QB], f32, tag="pscr", name="pscr")
                    mm(pscr, kT[h][:, kt * P:(kt + 1) * P],
                       qT[h][:, qsl], True, True)
                    et = epool.tile([P, QB], f16, tag="et", name="et")
                    nc.scalar.activation(et[:], pscr[:], Act.Exp,
                                         scale=float(SCALE))
                    j = kt - qb * (QB // P)
                    if j >= 0:
                        nc.vector.tensor_tensor(
                            et[:], et[:], masks[:, j, :], Alu.mult
                        )
                    if kt == 0:
                        nc.vector.tensor_copy(esum[:], et[:])
                    else:
                        nc.vector.tensor_tensor(esum[:], esum[:], et[:], Alu.add)
                    mm(po, vs[:, kt, h * P:(h + 1) * P], et[:],
                       kt == 0, kt == nkt - 1)
                # denominator: partition-reduce esum in a short-lived score
                # bank, then reciprocal
                pd = pss.tile([P, QB], f32, tag="pscr", name="pd")
                mm(pd, oneb[:], esum[:], True, True)
                rec = npool.tile([P, QB], f32, tag="rec", name="rec")
                nc.vector.reciprocal_approx_fast(rec[:], pd[:])
                nc.vector.tensor_tensor(
                    ao[h][:, qsl], po[:], rec[:], Alu.mult
                )

            # ---- heads 0-2: proj ring 3 + score ring 3 + out 2 = 8 ----
            with (
                tc.tile_pool(name="pss", bufs=3, space="PSUM") as pss,
                tc.tile_pool(name="pso", bufs=2, space="PSUM") as pso,
            ):
                with tc.tile_pool(name="psqk", bufs=1, space="PSUM") as psqk:
                    # k0/q0/v interleaved per x-chunk so the x stream never
                    # starves PE (pv shares the score-bank ring)
                    for g in range(4):
                        proj_head("k", 0, wk0, kT[0], psqk, tbs=[g])
                        proj_head("q", 0, wq0, qT[0], psqk, tbs=[g])
                        for tt in range(4 * g, 4 * g + 4):
                            pv = pss.tile([P, DSL], f32, tag="pscr", name="pv")
                            for ko in range(KO):
                                piece = xsl(ko, tt * P, (tt + 1) * P)
                                assert len(piece) == 1
                                mm(pv, piece[0], wv[:, ko, :],
                                   ko == 0, ko == KO - 1)
                            nc.scalar.copy(vs[:, tt, :], pv[:])

                    # per-head pipeline: attention(h) fills with proj(h+1)
                    for h in range(HPC - 1):
                        for qb in range(NQB):
                            attn_head_qb(h, qb, pss, pso)
                        wk = wph_load("k", h + 1)
                        proj_head("k", h + 1, wk, kT[h + 1], psqk)
                        wq = wph_load("q", h + 1)
                        proj_head("q", h + 1, wq, qT[h + 1], psqk)

            # ---- attn(3) interleaved with output projection ----
            # second-generation PSUM split: scores 2 + out 2 + y 4 = 8;
            # exp stays one query-block ahead of the oproj matmuls
            NEC = H // QB
            with (
                tc.tile_pool(name="pss2", bufs=2, space="PSUM") as pss2,
                tc.tile_pool(name="pso2", bufs=2, space="PSUM") as pso2,
                tc.tile_pool(name="psy", bufs=1, space="PSUM") as psy,
            ):
                def oproj_tile(tt):
                    tsl = slice(tt * P, (tt + 1) * P)
                    pys = [psy.tile([P, QB], f32, tag=f"py{ec}",
                                    name=f"py{ec}") for ec in range(NEC)]
                    for dc in range(DSL // P):
                        for ec in range(NEC):
                            mm(pys[ec], ao[dc][:, tsl],
                               wos[:, dc, ec * QB:(ec + 1) * QB],
                               dc == 0, dc == DSL // P - 1)
                    yst = ypool.tile([P, H], f16, tag="yst", name="yst")
                    for ec in range(NEC):
                        eng_copy = (nc.scalar.copy if ec % 2 == 0
                                    else nc.vector.tensor_copy)
                        eng_copy(yst[:, ec * QB:(ec + 1) * QB], pys[ec][:])
                    nc.sync.dma_start(yout[tsl, :], yst[:])

                attn_head_qb(HPC - 1, 0, pss2, pso2)
                for qb in range(1, NQB):
                    attn_head_qb(HPC - 1, qb, pss2, pso2)
                    for tt in range((qb - 1) * NQB, qb * NQB):
                        oproj_tile(tt)
                for tt in range((NQB - 1) * NQB, NQB * NQB):
                    oproj_tile(tt)

    nc.finalize()
    return nc


def _host_inputs(hidden_states, wq, wk, wv, wo):
    f32 = np.float32
    f16 = np.float16
    ca = np.ascontiguousarray

    inv = 1.0 / (ROPE_BASE ** (np.arange(0, HD, 2, dtype=f32) / HD))
    t = np.arange(S, dtype=f32)
    fr = np.outer(t, inv)                      # [S, 64]
    cosT = np.cos(fr).T.astype(f32)            # [64, S]
    sinT = np.sin(fr).T.astype(f32)
    csa = ca(np.concatenate([cosT, cosT], axis=0)).astype(f16)  # [128, S]
    csb = ca(np.concatenate([sinT, sinT], axis=0)).astype(f16)

    jj, pp, ff = np.meshgrid(
        np.arange(QB // P), np.arange(P), np.arange(QB), indexing="ij"
    )
    mask = np.where(jj * P + pp > ff, f16(0.0), f16(1.0)).astype(f16)
    onesb = np.ones((P, P), f16)

    xTb = [ca(hidden_states[b].T.astype(f16)) for b in range(B)]

    in_maps = []
    for c in range(NCORES):
        b, hg = divmod(c, NCORES // B)
        dsl = slice(hg * DSL, (hg + 1) * DSL)
        in_maps.append({
            "xT": xTb[b],
            "wqT": ca(wq[dsl, :].T.astype(f16)),
            "wkT": ca(wk[dsl, :].T.astype(f16)),
            "wvT": ca(wv[dsl, :].T.astype(f16)),
            "woT": ca(wo[:, dsl].T.astype(f16)),
            "csa": csa, "csb": csb,
            "mask": mask, "onesb": onesb,
        })
    return in_maps


def kernel(hidden_states, wq, wk, wv, wo, trace=False):
    from concourse.bass_utils import run_bass_kernel_spmd

    if "nc" not in _CACHE:
        _CACHE["nc"] = _build_nc()
    nc = _CACHE["nc"]

    in_maps = _host_inputs(
        np.asarray(hidden_states), np.asarray(wq), np.asarray(wk),
        np.asarray(wv), np.asarray(wo),
    )
    res = run_bass_kernel_spmd(nc, in_maps, core_ids=list(range(NCORES)),
                               trace=trace)
    y = np.zeros((B, S, H), np.float32)
    for c in range(NCORES):
        y[c // (NCORES // B)] += res.results[c]["out"].astype(np.float32)
    if trace:
        return y, res
    return y


# revision 25
# speedup vs baseline: 1.3702x; 1.0046x over previous
"""Self-contained Trainium2 kernel for nn_AMDOptimizedAttention.

Reference computes, for B=2, S=2048, H=2048, nh=16, hd=128:
    q/k/v = hs @ w{q,k,v}.T  (torch Linear convention)
    q, k  = rope(q), rope(k)
    out   = causal_softmax(q @ k.T / sqrt(hd)) @ v
    y     = out @ wo.T

Sharding (Megatron-style tensor parallel over heads + data parallel over
batch): core c handles batch c//4, heads 4*(c%4) .. 4*(c%4)+3.  Each core
computes a partial y for its batch (row-sharded wo); host sums the 4
partials per batch.

v6 layout: fp16 staging everywhere (same PE speed as bf16, 8x the
mantissa), f32 PSUM accumulation.
  - scores computed transposed [k, q]; causal mask applied as a 0/1
    fp16 multiply AFTER exp (2x DVE mode, off the PSUM critical path);
    softmax denominator: et tiles accumulated elementwise on DVE (fp16
    2x) into esum, ONE ones-matmul per (h, qb) partition-reduces it;
    1/sum via reciprocal_approx_fast.
  - per-head software pipeline: attention(h) emitted before proj(h+1);
    the tile scheduler fills exp-latency stalls on PE with projection
    matmuls.  PSUM: proj ring 3 + score/denom ring 3 + attn-out 2 = 8.
  - attention(3) runs interleaved with the output projection under a
    second-generation PSUM split (scores 2 + out 2 + y 4), staggered so
    exp stays one query-block ahead of the oproj matmuls.
  - k0/q0/v startup is interleaved per x-chunk (first chunk split in
    half) so the x DMA stream never starves PE.
"""

import sys

if "/opt/trn_rl_repo" not in sys.path:
    sys.path.insert(0, "/opt/trn_rl_repo")

import numpy as np

B, S, H = 2, 2048, 2048
NH, HD = 16, 128
P = 128
NCORES = 8
HPC = 4              # heads per core
DSL = HPC * HD       # 512: per-core slice of the hidden dim
KO = H // P          # 16 contraction chunks for projections
TBP = 512            # projection token-block
QB = 512             # attention query-block
NQB = S // QB        # 4
SCALE = 1.0 / np.sqrt(HD)
ROPE_BASE = 10000.0

_CACHE = {}


def _build_nc():
    import concourse.mybir as mybir
    from concourse import bacc
    from concourse.tile import TileContext

    f32 = mybir.dt.float32
    f16 = mybir.dt.float16
    Alu = mybir.AluOpType
    Act = mybir.ActivationFunctionType

    nc = bacc.Bacc("TRN2", target_bir_lowering=False)

    xT = nc.declare_dram_parameter("xT", [H, S], f16, isOutput=False)
    wqT = nc.declare_dram_parameter("wqT", [H, DSL], f16, isOutput=False)
    wkT = nc.declare_dram_parameter("wkT", [H, DSL], f16, isOutput=False)
    wvT = nc.declare_dram_parameter("wvT", [H, DSL], f16, isOutput=False)
    woT = nc.declare_dram_parameter("woT", [DSL, H], f16, isOutput=False)
    # rope tables packed [128, S]: rows 0:64 cos, rows 64:128 cos (dup);
    # csb likewise for sin
    csa = nc.declare_dram_parameter("csa", [P, S], f16, isOutput=False)
    csb = nc.declare_dram_parameter("csb", [P, S], f16, isOutput=False)
    maskp = nc.declare_dram_parameter("mask", [QB // P, P, QB], f16, isOutput=False)
    onesb = nc.declare_dram_parameter("onesb", [P, P], f16, isOutput=False)
    yout = nc.declare_dram_parameter("out", [S, H], f16, isOutput=True)

    xTr = xT.rearrange("(ko p) t -> p ko t", p=P)
    wT = {"q": wqT, "k": wkT, "v": wvT}
    wTr = {k: v.rearrange("(ko p) d -> p ko d", p=P) for k, v in wT.items()}

    def mm(ps, lhsT, rhs, start, stop):
        nc.tensor.matmul(ps, lhsT, rhs, start=start, stop=stop)

    with TileContext(nc) as tc, nc.allow_low_precision(
        reason="fp16 staging is deliberate; matmuls accumulate in f32 PSUM"
    ):
        with (
            tc.tile_pool(name="res", bufs=1) as rpool,
            tc.tile_pool(name="xres", bufs=1) as xpool,
            tc.tile_pool(name="wvpool", bufs=1) as wvpool,
            tc.tile_pool(name="wstream", bufs=2) as wpool,
            tc.tile_pool(name="ropetmp", bufs=2) as rtpool,
            tc.tile_pool(name="et", bufs=5) as epool,
            tc.tile_pool(name="nrm", bufs=2) as npool,
        ):
            # ---- residents (DMA issue order = priority order) ----
            TC = S // 4
            HC = TC // 2
            xs = [xpool.tile([P, KO, TC], f16, tag=f"xs{g}",
                             name=f"xs{g}") for g in range(4)]
            qT = [rpool.tile([P, S], f16, tag=f"qT{h}", name=f"qT{h}")
                  for h in range(HPC)]
            kT = [rpool.tile([P, S], f16, tag=f"kT{h}", name=f"kT{h}")
                  for h in range(HPC)]
            vs = rpool.tile([P, KO, DSL], f16, tag="vs", name="vs")
            ao = [rpool.tile([P, S], f16, tag=f"ao{h}", name=f"ao{h}")
                  for h in range(HPC)]
            wv = wvpool.tile([P, KO, DSL], f16, tag="wv", name="wv")

            def wph_load(pj, h):
                t = wpool.tile([P, KO, P], f16, tag="wph", name=f"w{pj}{h}")
                nc.sync.dma_start(t[:], wTr[pj][:, :, h * P:(h + 1) * P])
                return t

            # head-0 weights + first x chunk first, wv before the x tail
            # so v-proj matmuls can fill x-stream stalls
            wk0 = wph_load("k", 0)
            nc.sync.dma_start(xs[0][:], xTr[:, :, 0:TC])
            wq0 = wph_load("q", 0)
            nc.sync.dma_start(wv[:], wTr["v"][:])
            for g in range(1, 4):
                nc.sync.dma_start(xs[g][:], xTr[:, :, g * TC:(g + 1) * TC])
            csA = rpool.tile([P, S], f16, tag="csA", name="csA")
            nc.sync.dma_start(csA[:], csa[:])
            csB = rpool.tile([P, S], f16, tag="csB", name="csB")
            nc.sync.dma_start(csB[:], csb[:])
            masks = rpool.tile([P, QB // P, QB], f16, tag="masks", name="masks")
            nc.sync.dma_start(masks[:], maskp.rearrange("j p f -> p j f"))
            oneb = rpool.tile([P, P], f16, tag="oneb", name="oneb")
            nc.sync.dma_start(oneb[:], onesb[:])
            wos = rpool.tile([P, DSL // P, H], f16, tag="wos", name="wos")
            nc.sync.dma_start(wos[:], woT.rearrange("(ko p) e -> p ko e", p=P))

            def xsl(ko, t0, t1):
                # [t0, t1) slice of x.T for contraction chunk ko (single
                # 512-token chunk)
                g = t0 // TC
                assert t1 <= (g + 1) * TC
                return xs[g][:, ko, t0 - g * TC:t1 - g * TC]

            # ---- k/q projection, one token-block at a time ----
            NTB = S // TBP

            def proj_head(pj, h, wph, dst, psqk, tbs=None):
                for tb in (range(NTB) if tbs is None else tbs):
                    blocks = [(tb * TBP, (tb + 1) * TBP)]
                    for t0, t1 in blocks:
                        w = t1 - t0
                        ps = psqk.tile([P, TBP], f32, tag="psqk",
                                       name="psqk", bufs=3)
                        for ko in range(KO):
                            mm(ps[:, 0:w], wph[:, ko, :], xsl(ko, t0, t1),
                               ko == 0, ko == KO - 1)
                        tslc = slice(t0, t1)
                        psw = ps[:, 0:w]
                        # m1 = [x1*cos; x2*cos], m2 = [x2*sin; x1*sin]
                        # (halves swapped at creation: PSUM source is exempt
                        # from the same-start-partition rule, SBUF operands
                        # stay aligned)
                        m1 = rtpool.tile([P, TBP], f16, tag="m1", name="m1")
                        m2 = rtpool.tile([P, TBP], f16, tag="m2", name="m2")
                        nc.vector.tensor_tensor(
                            m1[:, 0:w], psw, csA[:, tslc], Alu.mult)
                        nc.vector.tensor_tensor(
                            m2[0:64, 0:w], ps[64:128, 0:w], csB[0:64, tslc],
                            Alu.mult)
                        nc.vector.tensor_tensor(
                            m2[64:128, 0:w], ps[0:64, 0:w], csB[64:128, tslc],
                            Alu.mult)
                        nc.vector.tensor_tensor(
                            dst[0:64, tslc], m1[0:64, 0:w], m2[0:64, 0:w],
                            Alu.subtract)
                        nc.vector.tensor_tensor(
                            dst[64:128, tslc], m1[64:128, 0:w],
                            m2[64:128, 0:w], Alu.add)

            # ---- attention for one (head, query-block) ----
            def attn_head_qb(h, qb, pss, pso):
                qsl = slice(qb * QB, (qb + 1) * QB)
                nkt = (qb + 1) * (QB // P)
                po = pso.tile([P, QB], f32, tag="po", name="po")
                esum = npool.tile([P, QB], f16, tag="esum", name="esum", bufs=2)
                for kt in range(nkt):
                    pscr = pss.tile([P, QB], f32, tag="pscr", name="pscr")
                    mm(pscr, kT[h][:, kt * P:(kt + 1) * P],
                       qT[h][:, qsl], True, True)
                    et = epool.tile([P, QB], f16, tag="et", name="et")
                    nc.scalar.activation(et[:], pscr[:], Act.Exp,
                                         scale=float(SCALE))
                    j = kt - qb * (QB // P)
                    if j >= 0:
                        nc.vector.tensor_tensor(
                            et[:], et[:], masks[:, j, :], Alu.mult
                        )
                    if kt == 0:
                        nc.vector.tensor_copy(esum[:], et[:])
                    else:
                        nc.vector.tensor_tensor(esum[:], esum[:], et[:], Alu.add)
                    mm(po, vs[:, kt, h * P:(h + 1) * P], et[:],
                       kt == 0, kt == nkt - 1)
                # denominator: partition-reduce esum in a short-lived score
                # bank, then reciprocal
                pd = pss.tile([P, QB], f32, tag="pscr", name="pd")
                mm(pd, oneb[:], esum[:], True, True)
                rec = npool.tile([P, QB], f32, tag="rec", name="rec")
                nc.vector.reciprocal_approx_fast(rec[:], pd[:])
                nc.vector.tensor_tensor(
                    ao[h][:, qsl], po[:], rec[:], Alu.mult
                )

            # ---- heads 0-2: proj ring 3 + score ring 3 + out 2 = 8 ----
            with (
                tc.tile_pool(name="pss", bufs=3, space="PSUM") as pss,
                tc.tile_pool(name="pso", bufs=2, space="PSUM") as pso,
            ):
                with tc.tile_pool(name="psqk", bufs=1, space="PSUM") as psqk:
                    # k0/q0/v interleaved per x-chunk so the x stream never
                    # starves PE (pv shares the score-bank ring)
                    for g in range(4):
                        proj_head("k", 0, wk0, kT[0], psqk, tbs=[g])
                        proj_head("q", 0, wq0, qT[0], psqk, tbs=[g])
                        for tt in range(4 * g, 4 * g + 4):
                            pv = pss.tile([P, DSL], f32, tag="pscr", name="pv")
                            for ko in range(KO):
                                mm(pv, xsl(ko, tt * P, (tt + 1) * P),
                                   wv[:, ko, :], ko == 0, ko == KO - 1)
                            nc.scalar.copy(vs[:, tt, :], pv[:])

                    # per-head pipeline: attention(h) fills with proj(h+1)
                    for h in range(HPC - 1):
                        for qb in range(NQB):
                            attn_head_qb(h, qb, pss, pso)
                        wk = wph_load("k", h + 1)
                        proj_head("k", h + 1, wk, kT[h + 1], psqk)
                        wq = wph_load("q", h + 1)
                        proj_head("q", h + 1, wq, qT[h + 1], psqk)

            # ---- attn(3) interleaved with output projection ----
            # second-generation PSUM split: scores 2 + out 2 + y 4 = 8;
            # exp stays one query-block ahead of the oproj matmuls
            NEC = H // QB
            with (
                tc.tile_pool(name="pss2", bufs=2, space="PSUM") as pss2,
                tc.tile_pool(name="pso2", bufs=2, space="PSUM") as pso2,
                tc.tile_pool(name="psy", bufs=1, space="PSUM") as psy,
                tc.tile_pool(name="ystage", bufs=2) as ypool,
            ):
                def oproj_tile(tt):
                    tsl = slice(tt * P, (tt + 1) * P)
                    pys = [psy.tile([P, QB], f32, tag=f"py{ec}",
                                    name=f"py{ec}") for ec in range(NEC)]
                    for dc in range(DSL // P):
                        for ec in range(NEC):
                            mm(pys[ec], ao[dc][:, tsl],
                               wos[:, dc, ec * QB:(ec + 1) * QB],
                               dc == 0, dc == DSL // P - 1)
                    yst = ypool.tile([P, H], f16, tag="yst", name="yst")
                    for ec in range(NEC):
                        eng_copy = (nc.scalar.copy if ec % 2 == 0
                                    else nc.vector.tensor_copy)
                        eng_copy(yst[:, ec * QB:(ec + 1) * QB], pys[ec][:])
                    nc.sync.dma_start(yout[tsl, :], yst[:])

                attn_head_qb(HPC - 1, 0, pss2, pso2)
                for qb in range(1, NQB):
                    attn_head_qb(HPC - 1, qb, pss2, pso2)
                    for tt in range((qb - 1) * NQB, qb * NQB):
                        oproj_tile(tt)
                for tt in range((NQB - 1) * NQB, NQB * NQB):
                    oproj_tile(tt)

    nc.finalize()
    return nc


def _host_inputs(hidden_states, wq, wk, wv, wo):
    f32 = np.float32
    f16 = np.float16
    ca = np.ascontiguousarray

    inv = 1.0 / (ROPE_BASE ** (np.arange(0, HD, 2, dtype=f32) / HD))
    t = np.arange(S, dtype=f32)
    fr = np.outer(t, inv)                      # [S, 64]
    cosT = np.cos(fr).T.astype(f32)            # [64, S]
    sinT = np.sin(fr).T.astype(f32)
    csa = ca(np.concatenate([cosT, cosT], axis=0)).astype(f16)  # [128, S]
    csb = ca(np.concatenate([sinT, sinT], axis=0)).astype(f16)

    jj, pp, ff = np.meshgrid(
        np.arange(QB // P), np.arange(P), np.arange(QB), indexing="ij"
    )
    mask = np.where(jj * P + pp > ff, f16(0.0), f16(1.0)).astype(f16)
    onesb = np.ones((P, P), f16)

    xTb = [ca(hidden_states[b].T.astype(f16)) for b in range(B)]

    in_maps = []
    for c in range(NCORES):
        b, hg = divmod(c, NCORES // B)
        dsl = slice(hg * DSL, (hg + 1) * DSL)
        in_maps.append({
            "xT": xTb[b],
            "wqT": ca(wq[dsl, :].T.astype(f16)),
            "wkT": ca(wk[dsl, :].T.astype(f16)),
            "wvT": ca(wv[dsl, :].T.astype(f16)),
            "woT": ca(wo[:, dsl].T.astype(f16)),
            "csa": csa, "csb": csb,
            "mask": mask, "onesb": onesb,
        })
    return in_maps


def kernel(hidden_states, wq, wk, wv, wo, trace=False):
    from concourse.bass_utils import run_bass_kernel_spmd

    if "nc" not in _CACHE:
        _CACHE["nc"] = _build_nc()
    nc = _CACHE["nc"]

    in_maps = _host_inputs(
        np.asarray(hidden_states), np.asarray(wq), np.asarray(wk),
        np.asarray(wv), np.asarray(wo),
    )
    res = run_bass_kernel_spmd(nc, in_maps, core_ids=list(range(NCORES)),
                               trace=trace)
    y = np.zeros((B, S, H), np.float32)
    for c in range(NCORES):
        y[c // (NCORES // B)] += res.results[c]["out"].astype(np.float32)
    if trace:
        return y, res
    return y


# revision 27
# speedup vs baseline: 1.4158x; 1.0332x over previous
"""Self-contained Trainium2 kernel for nn_AMDOptimizedAttention.

Reference computes, for B=2, S=2048, H=2048, nh=16, hd=128:
    q/k/v = hs @ w{q,k,v}.T  (torch Linear convention)
    q, k  = rope(q), rope(k)
    out   = causal_softmax(q @ k.T / sqrt(hd)) @ v
    y     = out @ wo.T

Sharding (Megatron-style tensor parallel over heads + data parallel over
batch): core c handles batch c//4, heads 4*(c%4) .. 4*(c%4)+3.  Each core
computes a partial y for its batch (row-sharded wo); host sums the 4
partials per batch.

v6 layout: fp16 staging everywhere (same PE speed as bf16, 8x the
mantissa), f32 PSUM accumulation.
  - scores computed transposed [k, q]; causal mask applied as a 0/1
    fp16 multiply AFTER exp (2x DVE mode, off the PSUM critical path);
    softmax denominator: et tiles accumulated elementwise on DVE (fp16
    2x) into esum, ONE ones-matmul per (h, qb) partition-reduces it;
    1/sum via reciprocal_approx_fast.
  - per-head software pipeline: attention(h) emitted before proj(h+1);
    the tile scheduler fills exp-latency stalls on PE with projection
    matmuls.  PSUM: proj ring 3 + score/denom ring 3 + attn-out 2 = 8.
  - attention(3) runs interleaved with the output projection under a
    second-generation PSUM split (scores 2 + out 2 + y 4), staggered so
    exp stays one query-block ahead of the oproj matmuls.
  - k0/q0/v startup is interleaved per x-chunk (first chunk split in
    half) so the x DMA stream never starves PE.
"""

import sys

if "/opt/trn_rl_repo" not in sys.path:
    sys.path.insert(0, "/opt/trn_rl_repo")

import numpy as np

B, S, H = 2, 2048, 2048
NH, HD = 16, 128
P = 128
NCORES = 8
HPC = 4              # heads per core
DSL = HPC * HD       # 512: per-core slice of the hidden dim
KO = H // P          # 16 contraction chunks for projections
TBP = 512            # projection token-block
QB = 512             # attention query-block
NQB = S // QB        # 4
SCALE = 1.0 / np.sqrt(HD)
ROPE_BASE = 10000.0

_CACHE = {}


def _build_nc():
    import concourse.mybir as mybir
    from concourse import bacc
    from concourse.tile import TileContext

    f32 = mybir.dt.float32
    f16 = mybir.dt.float16
    Alu = mybir.AluOpType
    Act = mybir.ActivationFunctionType

    nc = bacc.Bacc("TRN2", target_bir_lowering=False)

    xT = nc.declare_dram_parameter("xT", [H, S], f16, isOutput=False)
    wqT = nc.declare_dram_parameter("wqT", [H, DSL], f16, isOutput=False)
    wkT = nc.declare_dram_parameter("wkT", [H, DSL], f16, isOutput=False)
    wvT = nc.declare_dram_parameter("wvT", [H, DSL], f16, isOutput=False)
    woT = nc.declare_dram_parameter("woT", [DSL, H], f16, isOutput=False)
    # rope tables packed [128, S]: rows 0:64 cos, rows 64:128 cos (dup);
    # csb likewise for sin
    csa = nc.declare_dram_parameter("csa", [P, S], f16, isOutput=False)
    csb = nc.declare_dram_parameter("csb", [P, S], f16, isOutput=False)
    maskp = nc.declare_dram_parameter("mask", [QB // P, P, QB], f16, isOutput=False)
    onesb = nc.declare_dram_parameter("onesb", [P, P], f16, isOutput=False)
    yout = nc.declare_dram_parameter("out", [S, H], f16, isOutput=True)

    xTr = xT.rearrange("(ko p) t -> p ko t", p=P)
    wT = {"q": wqT, "k": wkT, "v": wvT}
    wTr = {k: v.rearrange("(ko p) d -> p ko d", p=P) for k, v in wT.items()}

    def mm(ps, lhsT, rhs, start, stop):
        nc.tensor.matmul(ps, lhsT, rhs, start=start, stop=stop)

    with TileContext(nc) as tc, nc.allow_low_precision(
        reason="fp16 staging is deliberate; matmuls accumulate in f32 PSUM"
    ):
        with (
            tc.tile_pool(name="res", bufs=1) as rpool,
            tc.tile_pool(name="xres", bufs=1) as xpool,
            tc.tile_pool(name="wvpool", bufs=1) as wvpool,
            tc.tile_pool(name="wstream", bufs=2) as wpool,
            tc.tile_pool(name="ropetmp", bufs=2) as rtpool,
            tc.tile_pool(name="et", bufs=5) as epool,
            tc.tile_pool(name="nrm", bufs=2) as npool,
        ):
            # ---- residents (DMA issue order = priority order) ----
            TC = S // 4
            HC = TC // 2
            xs = [xpool.tile([P, KO, TC], f16, tag=f"xs{g}",
                             name=f"xs{g}") for g in range(4)]
            qT = [rpool.tile([P, S], f16, tag=f"qT{h}", name=f"qT{h}")
                  for h in range(HPC)]
            kT = [rpool.tile([P, S], f16, tag=f"kT{h}", name=f"kT{h}")
                  for h in range(HPC)]
            vs = rpool.tile([P, KO, DSL], f16, tag="vs", name="vs")
            ao = [rpool.tile([P, S], f16, tag=f"ao{h}", name=f"ao{h}")
                  for h in range(HPC)]
            wv = wvpool.tile([P, KO, DSL], f16, tag="wv", name="wv")

            def wph_load(pj, h):
                t = wpool.tile([P, KO, P], f16, tag="wph", name=f"w{pj}{h}")
                nc.sync.dma_start(t[:], wTr[pj][:, :, h * P:(h + 1) * P])
                return t

            # head-0 weights + first x chunk first, wv before the x tail
            # so v-proj matmuls can fill x-stream stalls
            wk0 = wph_load("k", 0)
            nc.sync.dma_start(xs[0][:], xTr[:, :, 0:TC])
            wq0 = wph_load("q", 0)
            nc.sync.dma_start(wv[:], wTr["v"][:])
            for g in range(1, 4):
                nc.sync.dma_start(xs[g][:], xTr[:, :, g * TC:(g + 1) * TC])
            csA = rpool.tile([P, S], f16, tag="csA", name="csA")
            nc.sync.dma_start(csA[:], csa[:])
            csB = rpool.tile([P, S], f16, tag="csB", name="csB")
            nc.sync.dma_start(csB[:], csb[:])
            masks = rpool.tile([P, QB // P, QB], f16, tag="masks", name="masks")
            nc.sync.dma_start(masks[:], maskp.rearrange("j p f -> p j f"))
            oneb = rpool.tile([P, P], f16, tag="oneb", name="oneb")
            nc.sync.dma_start(oneb[:], onesb[:])
            wos = rpool.tile([P, DSL // P, H], f16, tag="wos", name="wos")
            nc.sync.dma_start(wos[:], woT.rearrange("(ko p) e -> p ko e", p=P))

            def xsl(ko, t0, t1):
                # [t0, t1) slice of x.T for contraction chunk ko (single
                # 512-token chunk)
                g = t0 // TC
                assert t1 <= (g + 1) * TC
                return xs[g][:, ko, t0 - g * TC:t1 - g * TC]

            # ---- k/q projection, one token-block at a time ----
            NTB = S // TBP

            def proj_head(pj, h, wph, dst, psqk, tbs=None):
                for tb in (range(NTB) if tbs is None else tbs):
                    blocks = [(tb * TBP, (tb + 1) * TBP)]
                    for t0, t1 in blocks:
                        w = t1 - t0
                        ps = psqk.tile([P, TBP], f32, tag="psqk",
                                       name="psqk", bufs=3)
                        for ko in range(KO):
                            mm(ps[:, 0:w], wph[:, ko, :], xsl(ko, t0, t1),
                               ko == 0, ko == KO - 1)
                        tslc = slice(t0, t1)
                        psw = ps[:, 0:w]
                        # m1 = [x1*cos; x2*cos], m2 = [x2*sin; x1*sin]
                        # (halves swapped at creation: PSUM source is exempt
                        # from the same-start-partition rule, SBUF operands
                        # stay aligned)
                        m1 = rtpool.tile([P, TBP], f16, tag="m1", name="m1")
                        m2 = rtpool.tile([P, TBP], f16, tag="m2", name="m2")
                        nc.vector.tensor_tensor(
                            m1[:, 0:w], psw, csA[:, tslc], Alu.mult)
                        nc.vector.tensor_tensor(
                            m2[0:64, 0:w], ps[64:128, 0:w], csB[0:64, tslc],
                            Alu.mult)
                        nc.vector.tensor_tensor(
                            m2[64:128, 0:w], ps[0:64, 0:w], csB[64:128, tslc],
                            Alu.mult)
                        nc.vector.tensor_tensor(
                            dst[0:64, tslc], m1[0:64, 0:w], m2[0:64, 0:w],
                            Alu.subtract)
                        nc.vector.tensor_tensor(
                            dst[64:128, tslc], m1[64:128, 0:w],
                            m2[64:128, 0:w], Alu.add)

            # ---- attention for one (head, query-block) ----
            def attn_head_qb(h, qb, pss, pso):
                nkt = (qb + 1) * (QB // P)
                po = pso.tile([P, QB], f32, tag="po", name="po")
                esum = npool.tile([P, QB], f16, tag="esum", name="esum", bufs=2)
                for kt in range(nkt):
                    # diagonal chunk j >= 1 has valid queries only at
                    # columns >= j*128: trim score/exp/mask/esum/pv to them
                    j = kt - qb * (QB // P)
                    qo = max(j, 0) * P
                    w = QB - qo
                    qsl = slice(qb * QB + qo, (qb + 1) * QB)
                    pscr = pss.tile([P, QB], f32, tag="pscr", name="pscr")
                    mm(pscr[:, 0:w], kT[h][:, kt * P:(kt + 1) * P],
                       qT[h][:, qsl], True, True)
                    et = epool.tile([P, QB], f16, tag="et", name="et")
                    nc.scalar.activation(et[:, 0:w], pscr[:, 0:w], Act.Exp,
                                         scale=float(SCALE))
                    if j >= 0:
                        nc.vector.tensor_tensor(
                            et[:, 0:w], et[:, 0:w], masks[:, j, qo:], Alu.mult
                        )
                    if kt == 0:
                        nc.vector.tensor_copy(esum[:], et[:])
                    else:
                        nc.vector.tensor_tensor(
                            esum[:, qo:], esum[:, qo:], et[:, 0:w], Alu.add)
                    nc.tensor.matmul(po[:, qo:], vs[:, kt, h * P:(h + 1) * P],
                                     et[:, 0:w], start=(kt == 0),
                                     stop=(kt == nkt - 1),
                                     skip_group_check=True)
                # denominator: partition-reduce esum in a short-lived score
                # bank, then reciprocal
                pd = pss.tile([P, QB], f32, tag="pscr", name="pd")
                mm(pd, oneb[:], esum[:], True, True)
                rec = npool.tile([P, QB], f32, tag="rec", name="rec")
                nc.vector.reciprocal_approx_fast(rec[:], pd[:])
                nc.vector.tensor_tensor(
                    ao[h][:, qb * QB:(qb + 1) * QB], po[:], rec[:], Alu.mult
                )

            # ---- heads 0-2: proj ring 3 + score ring 3 + out 2 = 8 ----
            with (
                tc.tile_pool(name="pss", bufs=3, space="PSUM") as pss,
                tc.tile_pool(name="pso", bufs=2, space="PSUM") as pso,
            ):
                with tc.tile_pool(name="psqk", bufs=1, space="PSUM") as psqk:
                    # k0/q0/v interleaved per x-chunk so the x stream never
                    # starves PE (pv shares the score-bank ring)
                    for g in range(4):
                        proj_head("k", 0, wk0, kT[0], psqk, tbs=[g])
                        proj_head("q", 0, wq0, qT[0], psqk, tbs=[g])
                        for tt in range(4 * g, 4 * g + 4):
                            pv = pss.tile([P, DSL], f32, tag="pscr", name="pv")
                            for ko in range(KO):
                                mm(pv, xsl(ko, tt * P, (tt + 1) * P),
                                   wv[:, ko, :], ko == 0, ko == KO - 1)
                            nc.scalar.copy(vs[:, tt, :], pv[:])

                    # per-head pipeline: attention(h) fills with proj(h+1)
                    for h in range(HPC - 1):
                        for qb in range(NQB):
                            attn_head_qb(h, qb, pss, pso)
                        wk = wph_load("k", h + 1)
                        proj_head("k", h + 1, wk, kT[h + 1], psqk)
                        wq = wph_load("q", h + 1)
                        proj_head("q", h + 1, wq, qT[h + 1], psqk)

            # ---- attn(3) interleaved with output projection ----
            # second-generation PSUM split: scores 2 + out 2 + y 4 = 8;
            # exp stays one query-block ahead of the oproj matmuls
            NEC = H // QB
            with (
                tc.tile_pool(name="pss2", bufs=2, space="PSUM") as pss2,
                tc.tile_pool(name="pso2", bufs=2, space="PSUM") as pso2,
                tc.tile_pool(name="psy", bufs=1, space="PSUM") as psy,
                tc.tile_pool(name="ystage", bufs=2) as ypool,
            ):
                def oproj_tile(tt):
                    tsl = slice(tt * P, (tt + 1) * P)
                    pys = [psy.tile([P, QB], f32, tag=f"py{ec}",
                                    name=f"py{ec}") for ec in range(NEC)]
                    for dc in range(DSL // P):
                        for ec in range(NEC):
                            mm(pys[ec], ao[dc][:, tsl],
                               wos[:, dc, ec * QB:(ec + 1) * QB],
                               dc == 0, dc == DSL // P - 1)
                    yst = ypool.tile([P, H], f16, tag="yst", name="yst")
                    for ec in range(NEC):
                        eng_copy = (nc.scalar.copy if ec % 2 == 0
                                    else nc.vector.tensor_copy)
                        eng_copy(yst[:, ec * QB:(ec + 1) * QB], pys[ec][:])
                    nc.sync.dma_start(yout[tsl, :], yst[:])

                attn_head_qb(HPC - 1, 0, pss2, pso2)
                for qb in range(1, NQB):
                    attn_head_qb(HPC - 1, qb, pss2, pso2)
                    for tt in range((qb - 1) * NQB, qb * NQB):
                        oproj_tile(tt)
                for tt in range((NQB - 1) * NQB, NQB * NQB):
                    oproj_tile(tt)

    nc.finalize()
    return nc


def _host_inputs(hidden_states, wq, wk, wv, wo):
    f32 = np.float32
    f16 = np.float16
    ca = np.ascontiguousarray

    inv = 1.0 / (ROPE_BASE ** (np.arange(0, HD, 2, dtype=f32) / HD))
    t = np.arange(S, dtype=f32)
    fr = np.outer(t, inv)                      # [S, 64]
    cosT = np.cos(fr).T.astype(f32)            # [64, S]
    sinT = np.sin(fr).T.astype(f32)
    csa = ca(np.concatenate([cosT, cosT], axis=0)).astype(f16)  # [128, S]
    csb = ca(np.concatenate([sinT, sinT], axis=0)).astype(f16)

    jj, pp, ff = np.meshgrid(
        np.arange(QB // P), np.arange(P), np.arange(QB), indexing="ij"
    )
    mask = np.where(jj * P + pp > ff, f16(0.0), f16(1.0)).astype(f16)
    onesb = np.ones((P, P), f16)

    xTb = [ca(hidden_states[b].T.astype(f16)) for b in range(B)]

    in_maps = []
    for c in range(NCORES):
        b, hg = divmod(c, NCORES // B)
        dsl = slice(hg * DSL, (hg + 1) * DSL)
        in_maps.append({
            "xT": xTb[b],
            "wqT": ca(wq[dsl, :].T.astype(f16)),
            "wkT": ca(wk[dsl, :].T.astype(f16)),
            "wvT": ca(wv[dsl, :].T.astype(f16)),
            "woT": ca(wo[:, dsl].T.astype(f16)),
            "csa": csa, "csb": csb,
            "mask": mask, "onesb": onesb,
        })
    return in_maps


def kernel(hidden_states, wq, wk, wv, wo, trace=False):
    from concourse.bass_utils import run_bass_kernel_spmd

    if "nc" not in _CACHE:
        _CACHE["nc"] = _build_nc()
    nc = _CACHE["nc"]

    in_maps = _host_inputs(
        np.asarray(hidden_states), np.asarray(wq), np.asarray(wk),
        np.asarray(wv), np.asarray(wo),
    )
    res = run_bass_kernel_spmd(nc, in_maps, core_ids=list(range(NCORES)),
                               trace=trace)
    y = np.zeros((B, S, H), np.float32)
    for c in range(NCORES):
        y[c // (NCORES // B)] += res.results[c]["out"].astype(np.float32)
    if trace:
        return y, res
    return y
